# revision 30
# baseline (speedup 1.0000x reference)
"""Trainium2 Bass/Tile kernel: AudioXMMDiT cross-attention, sharded over 8 NeuronCores.

Sharding: data-parallel over batch (2) x tensor-parallel over heads (4 groups of 8).
Each core computes, for its (batch b, heads h0..h0+7):
    q = x[b] @ Wq_c.T ; per-head RMSNorm
    k,v = context[b] @ {Wk_c,Wv_c}.T  (with the reference's cat(k,v)->(h d j)
          column interleave resolved on the host by permuting weight rows)
    out = softmax(q_n k_n^T / 8) @ v        -> out[b, :, h0*64:(h0+8)*64]

On-chip dataflow (all matmuls bf16 with fp32 PSUM accumulation):
    xT/ctxT/W*T arrive pre-transposed (contraction dim on partitions).
    Q is projected DIRECTLY in transposed (head-pair-major) layout: wq chunks
    are the PE-stationary operand and xT rows the moving operand, so qT
    [2*64 d, pair, n] falls out of PSUM with no PE transposes at all.
    Per-head sumsq comes from ACT Square (fp16) + a one-hot indicator matmul
    (E^T @ q^2 -> [4, 512]); rsqrt via Ln+Exp, with Exp/Ln/Square pinned to
    the natural_log_exp_and_others table set so the whole kernel does ONE
    ACT table load; the per-q scale is broadcast back across the wchunk's
    d-partitions by a K=4 matmul against F = E^T and applied with a single
    DVE multiply (psum x f32 -> bf16) that writes the scores-ready qT tile.
    K/V projections in natural layout (PE transposes for kT, once); scoresT
    via K=64 row-packed matmuls into a 2-deep ring of single-bank psum
    tiles, each drained by its own per-head ACT exp (FD=512) for fine-grain
    PE<->ACT decoupling; AV uses exp tiles as (FWL) stationary weights with
    v augmented by a ones column so the softmax denominator falls out of the
    same matmuls, head-group-major so the first exp piece is fully consumed
    halfway through AV and the next block's scores can recycle it early.

The q pipeline for block b+1 (16-MM projection quarters + norm chain) is
interleaved slot-by-slot with the attention of block b; exp tiles are split
per head-group (SBUF ring of 2) so successive blocks overlap. PSUM plan:
qt ring 3 + ss/bcast 1 + scores ring 2 + AV ring 2 = 8 banks exactly.
"""

import os
import sys
from contextlib import ExitStack

import numpy as np

for _p in ("/opt/trn_rl_repo",):
    if os.path.isdir(_p) and _p not in sys.path:
        sys.path.insert(0, _p)

import ml_dtypes  # noqa: E402

import concourse.bacc as bacc  # noqa: E402
import concourse.tile as tile  # noqa: E402
from concourse import bass_utils, hw_specs, mybir  # noqa: E402
from concourse.masks import make_identity  # noqa: E402


def _pin_act_table_set(arch):
    """Bind Exp/Ln/Square to the one table set that holds all three.

    The table-load placement pass assigns each ACTIVATE the first set in
    act_info.json containing its function (exp -> exp_and_others, ln ->
    natural_log), which makes a kernel using exp+ln toggle table sets every
    block (~2.6us per toggle). Pruning those functions from every other
    entry of the (cached) name->functions map makes the pass place a single
    natural_log_exp_and_others load for the whole kernel. Entry order (and
    therefore act_func_set_id numbering) is unchanged.
    """
    tabs = hw_specs.get_activation_tables(arch)
    pin = {mybir.ActivationFunctionType.Exp,
           mybir.ActivationFunctionType.Ln,
           mybir.ActivationFunctionType.Square}
    if pin <= tabs.get("natural_log_exp_and_others", set()):
        for name, fns in tabs.items():
            if name != "natural_log_exp_and_others":
                fns -= pin

P = 128
DIM = 2048
KC = DIM // P  # 16 contraction chunks
HK = KC // 2   # half of the contraction chunks (split-DMA granularity)
NH = 8         # heads per core
NPAIR = NH // 2
D = 64         # head dim
DA = D + 1     # + ones column (softmax denominator)
M = 512        # context length
MC = M // P    # kpos chunks
W = NH * D     # 512 output columns per core
EPS = 1e-6
SMSCALE = float(D) ** -0.5  # 1/8
NCORES = 8

BF = mybir.dt.bfloat16
F16 = mybir.dt.float16
F32 = mybir.dt.float32
AX = mybir.AxisListType
AF = mybir.ActivationFunctionType
MUL = mybir.AluOpType.mult
U32 = mybir.dt.uint32
SHR = mybir.AluOpType.logical_shift_right
XOR = mybir.AluOpType.bitwise_xor
SUB = mybir.AluOpType.subtract
ADD = mybir.AluOpType.add
# 0xFFFFFFFF - 0x5f3759df (so K - t == NOT(t) - this, avoiding reverse-subtract)
RSQRT_MAGIC_COMP = 0xFFFFFFFF - 0x5F3759DF

QB = 512
QCH = QB // P


def build_nc(n_q=4096):
    NQB = n_q // QB

    nc = bacc.Bacc(None, target_bir_lowering=False)
    _pin_act_table_set(nc.m.arch)

    xT = nc.dram_tensor("xT", (DIM, n_q), BF, kind="ExternalInput")
    ctxT = nc.dram_tensor("ctxT", (DIM, M), BF, kind="ExternalInput")
    wqT = nc.dram_tensor("wqT", (DIM, W), BF, kind="ExternalInput")
    wkT = nc.dram_tensor("wkT", (DIM, W), BF, kind="ExternalInput")
    wvT = nc.dram_tensor("wvT", (DIM, W), BF, kind="ExternalInput")
    out = nc.dram_tensor("out", (n_q, W), F32, kind="ExternalOutput")

    xT_r = xT[:].rearrange("(kc p) n -> p kc n", p=P)
    ctxT_r = ctxT[:].rearrange("(kc p) n -> p kc n", p=P)
    wqT_r = wqT[:].rearrange("(kc p) n -> p kc n", p=P)
    wkT_r = wkT[:].rearrange("(kc p) n -> p kc n", p=P)
    wvT_r = wvT[:].rearrange("(kc p) n -> p kc n", p=P)

    with tile.TileContext(nc) as tc, ExitStack() as es:
        consts = es.enter_context(tc.tile_pool(name="consts", bufs=1))
        stats = es.enter_context(tc.tile_pool(name="stats", bufs=3))
        # PSUM: qt ring 3 banks + ss/bcast 1 + scores (1x2 banks) + AV 2 = 8
        qtps = es.enter_context(tc.tile_pool(name="qtps", bufs=3, space="PSUM"))
        ssbps = es.enter_context(tc.tile_pool(name="ssbps", bufs=1, space="PSUM"))
        spool = es.enter_context(tc.tile_pool(name="spool", bufs=2, space="PSUM"))
        avpool = es.enter_context(tc.tile_pool(name="avpool", bufs=2, space="PSUM"))

        cst_sb = consts.tile([P, 2], F32)
        nc.vector.memset(cst_sb[:, 0:1], EPS)
        nc.vector.memset(cst_sb[:, 1:2], 0.0)
        eps_sb = cst_sb[:, 0:1]
        zero_sb = cst_sb[:, 1:2]

        wq_sb = consts.tile([P, KC, W], BF)
        kT_sb = consts.tile([P, NPAIR, M], BF)   # [pair-local 2*64, pair, kpos]
        v_sb = consts.tile([P, MC, NH, DA], BF)  # [kpos, mc, head, d + ones]
        nc.vector.memset(v_sb, 1.0)              # ones column; rest overwritten
        ident = consts.tile([P, P], F32)
        make_identity(nc, ident)

        # E[:, i, :]: [128 d-part, 4] one-hot for the in-group head of an
        # even (i=0) / odd (i=1) wchunk; F[0:4, i, :]: [4, 128] its transpose
        # (built by PE transpose: single-partition memsets are not 32-aligned).
        E32 = consts.tile([P, 2, 4], F32)
        nc.vector.memset(E32, 0.0)
        nc.vector.memset(E32[0:D, 0, 0:1], 1.0)
        nc.vector.memset(E32[D:P, 0, 1:2], 1.0)
        nc.vector.memset(E32[0:D, 1, 2:3], 1.0)
        nc.vector.memset(E32[D:P, 1, 3:4], 1.0)
        E_sb = consts.tile([P, 2, 4], F16)
        nc.vector.tensor_copy(out=E_sb, in_=E32)
        F_sb = consts.tile([P, 2, P], F16)

        xpool = es.enter_context(tc.tile_pool(name="xpool", bufs=6))
        qtpool = es.enter_context(tc.tile_pool(name="qtpool", bufs=2))
        opool = es.enter_context(tc.tile_pool(name="opool", bufs=4))
        epool = es.enter_context(tc.tile_pool(name="epool", bufs=2))
        x_tiles, qT_tiles, exp_tiles = {}, {}, {}
        qtp, sqd, rqd = {}, {}, {}

        def dve_rsqrt(pool, m_ss, scale, bias):
            """y = rsqrt(m_ss*scale + bias) entirely on DVE (no ACT tables):
            magic-constant seed + 2 Newton iterations, ~1e-5 rel err."""
            shp = list(m_ss.shape)
            m = pool.tile(shp, F32, tag="rsq_m")
            nc.vector.tensor_scalar(m, m_ss, scale, bias, MUL, ADD)
            y = pool.tile(shp, F32, tag="rsq_y")
            nc.vector.tensor_scalar(
                y.bitcast(U32), m.bitcast(U32), 1, 0xFFFFFFFF, SHR, XOR)
            nc.vector.tensor_scalar(
                y.bitcast(U32), y.bitcast(U32), RSQRT_MAGIC_COMP, None, SUB)
            t = pool.tile(shp, F32, tag="rsq_t")
            for _ in range(2):
                nc.vector.tensor_tensor(t, y, y, MUL)
                nc.vector.tensor_tensor(t, t, m, MUL)
                nc.vector.tensor_scalar(t, t, -0.5, 1.5, MUL, ADD)
                nc.vector.tensor_tensor(y, y, t, MUL)
            return y

        def load_x(b, engine=None):
            eng = engine or nc.gpsimd
            hs = []
            for h in range(2):
                t = xpool.tile([P, HK, QB], BF, name=f"x{b}_{h}", tag="x")
                eng.dma_start(
                    t, xT_r[:, h * HK:(h + 1) * HK, b * QB:(b + 1) * QB])
                hs.append(t)
            x_tiles[b] = hs

        def qproj_mms(b, w, kcs):
            """Direct-qT projection matmuls: wq chunk stationary, xT moving."""
            x_sb = x_tiles[b]
            qt = qtp[(b, w)]
            for kc in kcs:
                nc.tensor.matmul(
                    qt, wq_sb[:, kc, w * P:(w + 1) * P],
                    x_sb[kc // HK][:, kc % HK, :],
                    start=(kc == 0), stop=(kc == KC - 1))

        def qsquare(b, w):
            sq = stats.tile([P, QB], F16, tag="sq", bufs=4, name=f"sq{b}_{w}")
            nc.scalar.activation(sq, qtp[(b, w)], AF.Square, bias=zero_sb)
            sqd[(b, w)] = sq

        def qss_group(b, g):
            """ss rows 0:4 = per-head sumsq of the group's 4 heads (PE), then
            rq = exp(-0.5*ln(ss/64 + eps)) on ACT (one table set)."""
            ssp = ssbps.tile([P, QB], F32, tag="ssb", name=f"ss{b}_{g}")
            for i, w in enumerate((2 * g, 2 * g + 1)):
                nc.tensor.matmul(
                    ssp[0:4, :], E_sb[:, i, :], sqd.pop((b, w)),
                    start=(i == 0), stop=(i == 1))
            lnt = stats.tile([4, QB], F32, tag="lnt", bufs=2)
            nc.scalar.activation(
                lnt, ssp[0:4, :], AF.Ln, bias=eps_sb[0:4, :], scale=1.0 / D)
            rq = stats.tile([4, QB], F16, tag="rq", bufs=2, name=f"rq{b}_{g}")
            nc.scalar.activation(
                rq, lnt, AF.Exp, bias=zero_sb[0:4, :], scale=-0.5)
            rqd[(b, g)] = rq

        def qnorm_w(b, g, w):
            """Broadcast rq across the wchunk's 64 d-partitions (K=4 matmul)
            and apply: qT_sb[:, w, :] = qt_psum * rq_bcast (bf16 out)."""
            bcp = ssbps.tile([P, QB], F32, tag="ssb", name=f"bc{b}_{w}")
            nc.tensor.matmul(
                bcp, F_sb[0:4, w % 2, :], rqd[(b, g)], start=True, stop=True)
            bcs = stats.tile([P, QB], F32, tag="bcs", bufs=2)
            nc.vector.tensor_copy(out=bcs, in_=bcp)
            nc.vector.tensor_tensor(
                qT_tiles[b][:, w, :], qtp.pop((b, w)), bcs, MUL)

        def scores_pm(b, p, m):
            """scores^T for one head pair / kpos chunk: row-packed K=64 MMs
            into two single-bank psum tiles; per-head exp (1/8 scale) so the
            PE->ACT ring slack is one 570ns exp, not a whole-pair one."""
            qT_sb = qT_tiles[b]
            for half in range(2):
                lo = half * D
                sps = spool.tile([P, QB], F32, tag="sp",
                                 name=f"sps{p}_{m}_{half}")
                nc.tensor.matmul(
                    sps,
                    kT_sb[lo:lo + D, p, m * P:(m + 1) * P],
                    qT_sb[lo:lo + D, p, :],
                    start=True, stop=True)
                nc.scalar.activation(
                    exp_tiles[b][p // 2][:, (p % 2) * 2 + half, m, :], sps,
                    AF.Exp, bias=zero_sb, scale=SMSCALE)

        def av_phase(b):
            """AV for all 4 q-chunks, head-group-major: exp piece A (heads
            0-3) is fully consumed in the first half so the next block's
            pair-0/1 scores (which recycle that ring buffer) can start while
            piece B is still being read."""
            o_sbs = [opool.tile([P, NH, D], F32, name=f"o{qc}", tag="o")
                     for qc in range(4)]
            for hg in range(2):
                exp_sb = exp_tiles[b][hg]
                for qc in range(4):
                    avps = avpool.tile([P, 4, DA], F32, tag="av",
                                       name=f"av{qc}_{hg}")
                    for hh in range(4):
                        h = hg * 4 + hh
                        for mc in range(MC):
                            nc.tensor.matmul(
                                avps[:, hh, :],
                                exp_sb[:, hh, mc, qc * P:(qc + 1) * P],
                                v_sb[:, mc, h, :],
                                start=(mc == 0), stop=(mc == MC - 1))
                    rec = stats.tile([P, 4], F32, tag="rec")
                    nc.vector.reciprocal(rec, avps[:, :, D])
                    nc.vector.tensor_tensor(
                        o_sbs[qc][:, hg * 4:(hg + 1) * 4, :],
                        avps[:, :, 0:D],
                        rec[:, :, None].to_broadcast([P, 4, D]),
                        MUL)
                    if hg == 1:
                        nc.sync.dma_start(
                            out[b * QB + qc * P: b * QB + (qc + 1) * P, :],
                            o_sbs[qc][:].rearrange("p h d -> p (h d)"))

        # ---------------- Phase 1: K/V projections + block-0 q pipeline ----
        with tc.tile_pool(name="ph1", bufs=1) as ph1:
            ctx_h = [ph1.tile([P, HK, M], BF, name=f"ctx{h}") for h in range(2)]
            wk_h = [ph1.tile([P, HK, W], BF, name=f"wk{h}") for h in range(2)]
            wv_h = [ph1.tile([P, HK, W], BF, name=f"wv{h}") for h in range(2)]
            # Startup DMAs split into quarter-chunks: subtile deps let the
            # first K-projection matmuls start as soon as the first ~0.5MB
            # of ctx/wk lands instead of waiting for whole halves.
            HQ = HK // 2
            for j in range(2):
                ks = slice(j * HQ, (j + 1) * HQ)
                nc.gpsimd.dma_start(ctx_h[0][:, ks, :], ctxT_r[:, ks, :])
                nc.sync.dma_start(wk_h[0][:, ks, :], wkT_r[:, ks, :])
            nc.gpsimd.dma_start(wv_h[0], wvT_r[:, 0:HK, :])
            for j in range(2):
                ks = slice(j * HQ, (j + 1) * HQ)
                kg = slice(HK + j * HQ, HK + (j + 1) * HQ)
                nc.gpsimd.dma_start(ctx_h[1][:, ks, :], ctxT_r[:, kg, :])
                nc.sync.dma_start(wk_h[1][:, ks, :], wkT_r[:, kg, :])
            nc.sync.dma_start(wv_h[1], wvT_r[:, HK:KC, :])
            nc.gpsimd.dma_start(wq_sb, wqT_r)
            load_x(0, engine=nc.sync)

            for i in range(2):
                ftp = ssbps.tile([P, QB], F32, tag="ssb", name=f"ftp{i}")
                nc.tensor.transpose(ftp[0:4, 0:P], E32[:, i, :], ident)
                nc.vector.tensor_copy(out=F_sb[0:4, i, :], in_=ftp[0:4, 0:P])

            k_sb = ph1.tile([P, MC, W], F32)     # normalized k, natural layout
            exp_tiles[0] = [
                epool.tile([P, NH // 2, MC, QB], BF, name=f"exp0_{h}",
                           tag="exp") for h in range(2)]
            qT_tiles[0] = qtpool.tile([P, NPAIR, QB], BF, name="qT0", tag="qT")

            # K projection: 4 psum banks (3 from the qt ring + the ssb bank)
            kps_l = [qtps.tile([P, QB], F32, tag="qt", name=f"kps{m}")
                     for m in range(3)] + \
                    [ssbps.tile([P, QB], F32, tag="ssb", name="kps3")]
            for kc in range(KC):
                for mc in range(MC):
                    nc.tensor.matmul(
                        kps_l[mc],
                        ctx_h[kc // HK][:, kc % HK, mc * P:(mc + 1) * P],
                        wk_h[kc // HK][:, kc % HK, :],
                        start=(kc == 0), stop=(kc == KC - 1))

            for mc in range(MC):
                # V projection (natural layout) into a scores-ring bank
                vps = spool.tile([P, QB], F32, tag="sp", name=f"vps{mc}")
                for kc in range(KC):
                    nc.tensor.matmul(
                        vps,
                        ctx_h[kc // HK][:, kc % HK, mc * P:(mc + 1) * P],
                        wv_h[kc // HK][:, kc % HK, :],
                        start=(kc == 0), stop=(kc == KC - 1))
                nc.vector.tensor_copy(
                    out=v_sb[:, mc, :, 0:D],
                    in_=vps.rearrange("p (h d) -> p h d", h=NH))
                # k RMSNorm (natural layout): rk per (kpos, head) on DVE
                kps = kps_l[mc]
                sqk = stats.tile([P, W], F32, tag="ksq")
                nc.scalar.activation(sqk, kps, AF.Square, bias=zero_sb)
                ssk = stats.tile([P, NH], F32, tag="kss")
                nc.vector.reduce_sum(
                    ssk, sqk[:].rearrange("p (h d) -> p h d", h=NH), axis=AX.X)
                rk = dve_rsqrt(stats, ssk, 1.0 / D, EPS)
                nc.vector.tensor_tensor(
                    k_sb[:, mc, :].rearrange("p (h d) -> p h d", h=NH),
                    kps[:].rearrange("p (h d) -> p h d", h=NH),
                    rk[:, :, None].to_broadcast([P, NH, D]),
                    MUL)
                # block-0 q pipeline rides the freed kps banks (w0 at mc=2);
                # emitted before the transposes so the PE has work while the
                # DVE rsqrt/rescale chain for this mc drains.
                if mc >= 2:
                    w = mc - 2
                    qtp[(0, w)] = qtps.tile(
                        [P, QB], F32, tag="qt", name=f"qt0_{w}")
                    qproj_mms(0, w, range(KC))
                    qsquare(0, w)
                # kT via PE transpose-mode (once per kernel)
                tps = avpool.tile([P, NPAIR, P], F32, tag="av", name=f"tps{mc}")
                for pair in range(NPAIR):
                    nc.tensor.transpose(
                        tps[:, pair, :],
                        k_sb[:, mc, pair * P:(pair + 1) * P],
                        ident)
                nc.vector.tensor_copy(
                    out=kT_sb[:, :, mc * P:(mc + 1) * P], in_=tps)
                if mc == 1:
                    load_x(1)

            qss_group(0, 0)
            qnorm_w(0, 0, 0)
            for w in (2, 3):
                qtp[(0, w)] = qtps.tile([P, QB], F32, tag="qt", name=f"qt0_{w}")
                qproj_mms(0, w, range(KC))
                qsquare(0, w)
                if w == 2:
                    qnorm_w(0, 0, 1)
            qss_group(0, 1)
            qnorm_w(0, 1, 2)
            qnorm_w(0, 1, 3)

        # ---------------- Phase 2: software-pipelined main loop -----------
        for i in range(1, NQB + 1):
            bq = i if i < NQB else None      # block running its q pipeline
            ba = i - 1                       # block running attention
            if bq is not None:
                if bq + 1 < NQB:
                    load_x(bq + 1)
                exp_tiles[bq] = [
                    epool.tile([P, NH // 2, MC, QB], BF, name=f"exp{bq}_{h}",
                               tag="exp") for h in range(2)]
                qT_tiles[bq] = qtpool.tile(
                    [P, NPAIR, QB], BF, name=f"qT{bq}", tag="qT")
            for s in range(16):
                p, m = s // 4, s % 4
                scores_pm(ba, p, m)
                if bq is not None:
                    w, kq = s // 4, s % 4
                    if kq == 0:
                        qtp[(bq, w)] = qtps.tile(
                            [P, QB], F32, tag="qt", name=f"qt{bq}_{w}")
                    qproj_mms(bq, w, range(4 * kq, 4 * kq + 4))
                    if s == 3:
                        qsquare(bq, 0)
                    elif s == 7:
                        qsquare(bq, 1)
                        qss_group(bq, 0)
                    elif s == 10:
                        qnorm_w(bq, 0, 0)
                    elif s == 12:
                        qnorm_w(bq, 0, 1)
            # Phase B: AV + output for ba; norm tail for bq
            if bq is not None:
                qsquare(bq, 2)
                qsquare(bq, 3)
                qss_group(bq, 1)
                qnorm_w(bq, 1, 2)
                qnorm_w(bq, 1, 3)
            av_phase(ba)
            del x_tiles[ba], qT_tiles[ba], exp_tiles[ba]

        debug_pools = (consts, stats, xpool, qtpool, opool,
                       epool, qtps, ssbps, spool, avpool)

    if os.environ.get("KDEBUG_POOLS"):
        for pool in debug_pools:
            try:
                print(f"POOL {pool.name}: {pool.kb_per_partition_size()} KB/part"
                      f" bufs={pool.bufs} space={pool.space}")
                for k, meta in pool.tag_meta.items():
                    print("   ", k, meta)
            except Exception as e:
                print("POOL", pool.name, "err", e)

    nc.compile()
    return nc


_NC_CACHE = {}


def _get_nc(n_q=4096):
    if n_q not in _NC_CACHE:
        _NC_CACHE[n_q] = build_nc(n_q)
    return _NC_CACHE[n_q]


def make_in_maps(x, context, Wq, Wk, Wv):
    """Host-side shard + weight permutation. Returns one input map per core."""
    bf = ml_dtypes.bfloat16
    x = np.asarray(x)
    context = np.asarray(context)
    Wkv = np.concatenate([np.asarray(Wk), np.asarray(Wv)], axis=0)  # (4096, 2048)
    # reference: cat(k,v) reshaped (h d j): head h, dim d -> row h*128 + 2d (+1 for v)
    idx = np.arange(32)[:, None] * 128 + 2 * np.arange(64)[None, :]
    Wk_eff = Wkv[idx]       # (32, 64, 2048)
    Wv_eff = Wkv[idx + 1]   # (32, 64, 2048)
    Wq_eff = np.asarray(Wq).reshape(32, 64, 2048)

    xT = [np.ascontiguousarray(x[b].T).astype(bf) for b in range(x.shape[0])]
    ctxT = [np.ascontiguousarray(context[b].T).astype(bf)
            for b in range(context.shape[0])]

    in_maps = []
    for c in range(NCORES):
        b, hg = divmod(c, 4)
        hs = slice(hg * NH, (hg + 1) * NH)
        in_maps.append({
            "xT": xT[b],
            "ctxT": ctxT[b],
            "wqT": np.ascontiguousarray(
                Wq_eff[hs].reshape(W, DIM).T).astype(bf),
            "wkT": np.ascontiguousarray(
                Wk_eff[hs].reshape(W, DIM).T).astype(bf),
            "wvT": np.ascontiguousarray(
                Wv_eff[hs].reshape(W, DIM).T).astype(bf),
        })
    return in_maps


def assemble_output(results, n_q=4096, nb=2):
    outp = np.empty((nb, n_q, DIM), np.float32)
    for c in range(NCORES):
        b, hg = divmod(c, 4)
        outp[b, :, hg * W:(hg + 1) * W] = results[c]["out"]
    return outp


def kernel(x, context, Wq, Wk, Wv, **run_kwargs):
    nc = _get_nc(x.shape[1])
    in_maps = make_in_maps(x, context, Wq, Wk, Wv)
    res = bass_utils.run_bass_kernel_spmd(
        nc, in_maps, core_ids=list(range(NCORES)), **run_kwargs)
    out = assemble_output(res.results, n_q=x.shape[1], nb=x.shape[0])
    if run_kwargs:
        kernel.last_result = res
    return out
